# revision 7
# baseline (speedup 1.0000x reference)
"""GRU language-model forward on TRN2, 8 NeuronCores, batch-sharded.

Model (see reference):
    gx  = W_ihT[X] + b_ih                      # embedding-style row gather
    GRU scan over S=256 steps (PyTorch gate order r,z,n)
    out = Y @ W_dense.T + b_dense              # [S*B, V]

Sharding: data-parallel over batch. Each of the 8 cores owns 4 of the 32
sequences: it gathers its own gx rows, runs the full 256-step recurrence for
its 4 sequences, and computes + writes its [1024, 8192] slice of the output.
Weights are replicated.

On-chip layout ("T-layout"): the hidden state is kept transposed, G = 2*h as
a [128, 2*4] tile (partition = feature-half, free = (half, batch)).  The
factor 2 and the sigmoid/tanh identities below turn the whole gate chain into
tanh-only ACT ops plus fused scalar_tensor_tensor DVE ops:

    r  = sigmoid(ar)   = (1 + tanh(ar/2))/2
    z' = 1 - sigmoid(az) = (1 - tanh(az/2))/2
    n  = tanh(xn + r*hn)
    h' = n + z'*(h - n)   ==>   G' = 0.5*p - q,  p = (tz+1)*G,  q = (tz-1)*n

Host-side weight folds: W_hh and W_dense are pre-scaled by 0.5 (because the
stored state is G = 2h); b_ih+b_hh are folded into the gather table for the
r/z gates; b_ih only for n (b_hh_n must stay inside hn, preloaded into PSUM
via a rank-1 matmul).  x_r/x_z are preloaded into the PSUM accumulator banks
with PE transposes so the recurrent matmul lands directly on top of them.
"""

import sys

import numpy as np

for _p in ("/opt/trn_rl_repo",):
    if _p not in sys.path:
        sys.path.insert(0, _p)

V, H, B, S = 8192, 256, 32, 256
G3 = 3 * H          # 768
NCORES = 8
BL = B // NCORES    # 4 sequences per core
NBLK = 8            # token blocks
SPB = S // NBLK     # 32 steps per block
TOK = SPB * BL      # 128 tokens per block
ROWS = S * BL       # 1024 output rows per core
NV = V // 512       # 16 dense N-chunks

_cache = {}


def _build():
    import concourse.bass as bass
    import concourse.mybir as mybir
    import concourse.tile as tile
    from concourse.bacc import Bacc
    from concourse.masks import make_identity

    F32 = mybir.dt.float32
    I32 = mybir.dt.int32
    AOT = mybir.AluOpType
    AFT = mybir.ActivationFunctionType

    nc = Bacc()
    table = nc.declare_dram_parameter("table", [V, G3], F32, isOutput=False)
    whhT = nc.declare_dram_parameter("whhT", [H, G3], F32, isOutput=False)
    wdT = nc.declare_dram_parameter("wdT", [H, V], F32, isOutput=False)
    biasbc = nc.declare_dram_parameter("biasbc", [128, V], F32, isOutput=False)
    bhn = nc.declare_dram_parameter("bhn", [1, H], F32, isOutput=False)
    g0 = nc.declare_dram_parameter("g0", [128, 2 * BL], F32, isOutput=False)
    xidx = nc.declare_dram_parameter("xidx", [TOK, NBLK], I32, isOutput=False)
    y = nc.declare_dram_parameter("y", [ROWS, V], F32, isOutput=True)
    hout = nc.declare_dram_parameter("hout", [128, 2 * BL], F32, isOutput=True)

    with tile.TileContext(nc) as tc, \
            tc.tile_pool(name="const", bufs=1) as cp, \
            tc.tile_pool(name="gx", bufs=2) as gxp, \
            tc.tile_pool(name="st", bufs=3) as sp, \
            tc.tile_pool(name="prz", bufs=2, space="PSUM") as przp, \
            tc.tile_pool(name="pnx", bufs=2, space="PSUM") as pnxp, \
            tc.tile_pool(name="pd", bufs=2, space="PSUM") as pdp:

        ident = cp.tile([128, 128], F32, name="ident")
        make_identity(nc, ident[:])
        ones = cp.tile([1, 128], F32, name="ones")
        nc.vector.memset(ones[:], 1.0)

        idx_sb = cp.tile([TOK, NBLK], I32, name="idx")
        nc.sync.dma_start(out=idx_sb[:], in_=xidx[:, :])
        g0_sb = cp.tile([128, 2 * BL], F32, name="g0sb")
        nc.sync.dma_start(out=g0_sb[:], in_=g0[:, :])
        bhn_sb = cp.tile([1, H], F32, name="bhnsb")
        nc.sync.dma_start(out=bhn_sb[:], in_=bhn[:, :])
        whh_sb = cp.tile([128, 2 * G3], F32, name="whhsb")
        nc.sync.dma_start(out=whh_sb[:, 0:G3], in_=whhT[0:128, :])
        nc.sync.dma_start(out=whh_sb[:, G3:2 * G3], in_=whhT[128:256, :])
        yt = cp.tile([128, 2 * ROWS], F32, name="yt")
        wd_sb = cp.tile([128, 2 * V], F32, name="wdsb")
        nc.sync.dma_start(out=wd_sb[:, 0:V], in_=wdT[0:128, :])
        nc.sync.dma_start(out=wd_sb[:, V:2 * V], in_=wdT[128:256, :])
        bias_sb = cp.tile([128, V], F32, name="biassb")
        nc.sync.dma_start(out=bias_sb[:], in_=biasbc[:, :])

        yt2 = yt[:].rearrange("p (k c) -> p k c", k=2)
        g0v = g0_sb[:].rearrange("p (k c) -> p k c", k=2)

        def v2(t):
            return t[:].rearrange("p (k c) -> p k c", k=2)

        def gather(m):
            t = gxp.tile([TOK, G3], F32, tag="gx")
            nc.gpsimd.indirect_dma_start(
                out=t[:], out_offset=None, in_=table[:, :],
                in_offset=bass.IndirectOffsetOnAxis(ap=idx_sb[:, m:m + 1], axis=0))
            return t

        # One bank-preload instruction; idx 0..7, spread across late steps of
        # the previous block.  First writer of each bank carries start=True.
        def preload_inst(i, gx_t, rzb, nxb):
            if i < 4:      # x_r0, x_r1, x_z0, x_z1 transposed into rz bank
                nc.tensor.matmul(
                    out=rzb[:, i * 128:(i + 1) * 128],
                    lhsT=gx_t[:, i * 128:(i + 1) * 128], rhs=ident[:],
                    is_transpose=True, start=(i == 0), stop=False)
            elif i < 6:    # b_hh_n outer-product preload of n-halves
                hi = i - 4
                nc.tensor.matmul(
                    out=nxb[:, hi * 128:(hi + 1) * 128],
                    lhsT=bhn_sb[0:1, hi * 128:(hi + 1) * 128], rhs=ones[0:1, :],
                    start=(hi == 0), stop=False)
            else:          # x_n0, x_n1 transposed (read by the a_n op)
                hi = i - 6
                nc.tensor.matmul(
                    out=nxb[:, 256 + hi * 128:256 + (hi + 1) * 128],
                    lhsT=gx_t[:, 512 + hi * 128:512 + (hi + 1) * 128], rhs=ident[:],
                    is_transpose=True, start=False, stop=False)

        def dense_tile(mi, ni):
            ps = pdp.tile([128, 512], F32, tag="dps")
            for k in (0, 1):
                nc.tensor.matmul(
                    out=ps[:],
                    lhsT=yt[:, k * ROWS + mi * 128:k * ROWS + (mi + 1) * 128],
                    rhs=wd_sb[:, k * V + ni * 512:k * V + (ni + 1) * 512],
                    start=(k == 0), stop=(k == 1))
            ob = sp.tile([128, 512], F32, tag="ob", bufs=4)
            nc.vector.tensor_add(out=ob[:], in0=ps[:],
                                 in1=bias_sb[:, ni * 512:(ni + 1) * 512])
            nc.sync.dma_start(
                out=y[mi * 128:(mi + 1) * 128, ni * 512:(ni + 1) * 512], in_=ob[:])

        gx_t = gather(0)
        gx_next = None
        rzb = przp.tile([128, 512], F32, tag="rz")
        nxb = pnxp.tile([128, 512], F32, tag="nx")
        for i in range(8):
            preload_inst(i, gx_t, rzb, nxb)
        rz_nextb = nx_nextb = None

        # scan matmul emission order: r first (heads the chain), n next
        # (needed by u), z last (needed only at q).  (gate, bank, quarter)
        mm_plan = [(0, "rz", 0), (1, "rz", 128), (4, "nx", 0), (5, "nx", 128),
                   (2, "rz", 256), (3, "rz", 384)]

        for m in range(NBLK):
            if m + 1 < NBLK:
                gx_next = gather(m + 1)
            banks = {"rz": rzb, "nx": nxb}
            rz4 = rzb[:].rearrange("p (g c) -> p g c", g=4)
            nx4 = nxb[:].rearrange("p (g c) -> p g c", g=4)
            for sl in range(SPB):
                t = m * SPB + sl
                c4 = sl * BL
                gprev = g0v[:, :, :] if t == 0 else yt2[:, :, (t - 1) * BL:t * BL]
                last_step = sl == SPB - 1

                def rhs_k(k):
                    if t == 0:
                        return g0_sb[:, k * BL:(k + 1) * BL]
                    return yt[:, k * ROWS + (t - 1) * BL:k * ROWS + t * BL]

                for gi, bk, qoff in mm_plan:
                    for k in (0, 1):
                        # close each bank's accumulation group on its last MM
                        stop = last_step and k == 1 and gi in (3, 5)
                        nc.tensor.matmul(
                            out=banks[bk][:, qoff + c4:qoff + c4 + BL],
                            lhsT=whh_sb[:, k * G3 + gi * 128:k * G3 + (gi + 1) * 128],
                            rhs=rhs_k(k),
                            start=False, stop=stop)
                # interleave: dense output tiles for the previous block, and
                # PSUM-bank preloads for the next block
                if m >= 1 and sl % 2 == 0:
                    dense_tile(m - 1, sl // 2)
                if m + 1 < NBLK and sl >= SPB - 8:
                    i = sl - (SPB - 8)
                    if i == 0:
                        rz_nextb = przp.tile([128, 512], F32, tag="rz")
                        nx_nextb = pnxp.tile([128, 512], F32, tag="nx")
                    preload_inst(i, gx_next, rz_nextb, nx_nextb)

                tr = sp.tile([128, 2 * BL], F32, tag="tr")
                tz = sp.tile([128, 2 * BL], F32, tag="tz")
                u = sp.tile([128, 2 * BL], F32, tag="u")
                an = sp.tile([128, 2 * BL], F32, tag="an")
                nn = sp.tile([128, 2 * BL], F32, tag="nn")
                pp = sp.tile([128, 2 * BL], F32, tag="pp")
                qq = sp.tile([128, 2 * BL], F32, tag="qq")

                # tr = tanh(a_r/2); tz = tanh(a_z/2)
                nc.scalar.activation(out=v2(tr), in_=rz4[:, 0:2, c4:c4 + BL],
                                     func=AFT.Tanh, scale=0.5)
                nc.scalar.activation(out=v2(tz), in_=rz4[:, 2:4, c4:c4 + BL],
                                     func=AFT.Tanh, scale=0.5)
                # u = (tr + 1) * hn        (hn lives in PSUM)
                nc.vector.scalar_tensor_tensor(
                    out=v2(u), in0=v2(tr), scalar=1.0,
                    in1=nx4[:, 0:2, c4:c4 + BL], op0=AOT.add, op1=AOT.mult)
                # a_n = 0.5*u + x_n        (x_n lives in PSUM)
                nc.vector.scalar_tensor_tensor(
                    out=v2(an), in0=v2(u), scalar=0.5,
                    in1=nx4[:, 2:4, c4:c4 + BL], op0=AOT.mult, op1=AOT.add)
                # n = tanh(a_n)
                nc.scalar.activation(out=v2(nn), in_=v2(an), func=AFT.Tanh)
                # p = (tz + 1) * G_prev    (off critical path, overlaps tanh_n)
                nc.vector.scalar_tensor_tensor(
                    out=v2(pp), in0=v2(tz), scalar=1.0,
                    in1=gprev, op0=AOT.add, op1=AOT.mult)
                # q = (tz - 1) * n
                nc.vector.scalar_tensor_tensor(
                    out=v2(qq), in0=v2(tz), scalar=1.0,
                    in1=v2(nn), op0=AOT.subtract, op1=AOT.mult)
                # G' = 0.5*p - q  -> written straight into the Y buffer
                nc.vector.scalar_tensor_tensor(
                    out=yt2[:, :, t * BL:(t + 1) * BL], in0=v2(pp), scalar=0.5,
                    in1=v2(qq), op0=AOT.mult, op1=AOT.subtract)
            if m + 1 < NBLK:
                gx_t, gx_next = gx_next, None
                rzb, nxb = rz_nextb, nx_nextb

        for ni in range(NV):
            dense_tile(NBLK - 1, ni)

        ho = sp.tile([128, 2 * BL], F32, tag="ho")
        nc.vector.tensor_scalar_mul(
            v2(ho), yt2[:, :, (S - 1) * BL:S * BL], 0.5)
        nc.sync.dma_start(out=hout[:, :], in_=ho[:])

    return nc


def _prep_inputs(X, state, W_ih, W_hh, b_ih, b_hh, W_dense, b_dense):
    X = np.asarray(X).astype(np.int32)
    state = np.asarray(state, dtype=np.float32)
    W_ih = np.asarray(W_ih, dtype=np.float32)
    W_hh = np.asarray(W_hh, dtype=np.float32)
    b_ih = np.asarray(b_ih, dtype=np.float32)
    b_hh = np.asarray(b_hh, dtype=np.float32)
    W_dense = np.asarray(W_dense, dtype=np.float32)
    b_dense = np.asarray(b_dense, dtype=np.float32)

    # gather table: W_ih.T with the r/z biases (b_ih+b_hh) folded in, and only
    # b_ih for the n gate (b_hh_n is applied pre-r-multiply on device)
    table = W_ih.T.copy()
    bias_fold = b_ih.copy()
    bias_fold[0:2 * H] += b_hh[0:2 * H]
    table += bias_fold[None, :]
    table = np.ascontiguousarray(table, dtype=np.float32)

    whhT = np.ascontiguousarray(0.5 * W_hh.T, dtype=np.float32)        # [H, 3H]
    wdT = np.ascontiguousarray(0.5 * W_dense.T, dtype=np.float32)      # [H, V]
    biasbc = np.ascontiguousarray(
        np.broadcast_to(b_dense, (128, V)), dtype=np.float32)
    bhn = np.ascontiguousarray(b_hh[2 * H:3 * H][None, :], dtype=np.float32)

    in_maps = []
    for c in range(NCORES):
        bs = slice(BL * c, BL * (c + 1))
        # G0[f, 4*half + b] = 2*state[0, 4c+b, 128*half+f]
        st = state[0, bs, :].reshape(BL, 2, 128)          # [b, half, f]
        g0c = np.ascontiguousarray(
            (2.0 * st).transpose(2, 1, 0).reshape(128, 2 * BL), dtype=np.float32)
        # xidx[j, m] = X[4c + j%4, 32m + j//4]
        xc = X[bs, :]                                     # [BL, S]
        idx = np.empty((TOK, NBLK), dtype=np.int32)
        j = np.arange(TOK)
        for mth in range(NBLK):
            idx[:, mth] = xc[j % BL, SPB * mth + j // BL]
        in_maps.append(dict(table=table, whhT=whhT, wdT=wdT, biasbc=biasbc,
                            bhn=bhn, g0=g0c, xidx=idx))
    return in_maps


def kernel(X, state, W_ih, W_hh, b_ih, b_hh, W_dense, b_dense):
    from concourse.bass_utils import run_bass_kernel_spmd

    if "nc" not in _cache:
        nc = _build()
        if not nc.is_finalized():
            nc.finalize()
        _cache["nc"] = nc
    nc = _cache["nc"]

    in_maps = _prep_inputs(X, state, W_ih, W_hh, b_ih, b_hh, W_dense, b_dense)
    res = run_bass_kernel_spmd(nc, in_maps, list(range(NCORES)))

    out = np.empty((S * B, V), dtype=np.float32)
    h_last = np.empty((1, B, H), dtype=np.float32)
    outr = out.reshape(S, B, V)
    for c in range(NCORES):
        bs = slice(BL * c, BL * (c + 1))
        outr[:, bs, :] = res.results[c]["y"].reshape(S, BL, V)
        # hout[f, 4*half + b] -> h[4c+b, 128*half + f]
        ho = res.results[c]["hout"].reshape(128, 2, BL)
        h_last[0, bs, :] = ho.transpose(2, 1, 0).reshape(BL, H)
    return out, h_last
